# Initial kernel scaffold
#
"""ALSH Conv2d (LSH-routed subset conv) as a Bass/Tile kernel on 8 TRN2 NeuronCores.

Strategy (data-parallel over batch, per the sharding hint):
  * Each core gets 2 of the 16 images; kernels / a / table / the query patch
    (x[0,:,:2,:2]) are replicated.
  * On-device per core: hash the query via an interval-comparison table
    (no floor/mod chain), one-hot -> table row via two tiny matmuls,
    indirect-DMA gather of the 32 selected kernel rows (cast to bf16),
    PE-transpose into span-major GT inside one PSUM bank.
  * x is DMA'd contiguously, then reformatted on DVE/ACT into a zero-padded
    58x58 bf16 layout so conv im2col becomes pure access-pattern arithmetic
    on the matmul moving operand (no data movement).
  * Subset conv = 18 K-chunk matmuls per pass, 4-way column-tiled (PE array
    columns split over 4 row-tile groups) to recover throughput at M=32.
  * Output scale (O/BUCKET = 16) is folded into the PSUM->SBUF copies.
"""
import numpy as np

N_CORES = 8
_NC_CACHE = {}


def kernel(x, kernels, a, table):
    import concourse.bacc as bacc
    from concourse.bass_utils import run_bass_kernel_spmd

    x = np.ascontiguousarray(np.asarray(x, dtype=np.float32))
    kernels = np.ascontiguousarray(np.asarray(kernels, dtype=np.float32))
    a = np.ascontiguousarray(np.asarray(a, dtype=np.float32))
    table = np.ascontiguousarray(np.asarray(table, dtype=np.int32))

    per = x.shape[0] // N_CORES
    xq = np.ascontiguousarray(x[0, :, 0:2, 0:2])
    cand = _make_cand_table()
    in_maps = [
        {"xs": x[c * per:(c + 1) * per], "xq": xq, "av": a, "kern": kernels,
         "tab": table, "cand": cand}
        for c in range(N_CORES)
    ]

    if "nc" not in _NC_CACHE:
        nc = bacc.Bacc("TRN2", target_bir_lowering=False, debug=False)
        build(nc, mm_dtype="bf16", col_tile=True, xload="dve")
        nc.compile()
        _NC_CACHE["nc"] = nc
    nc = _NC_CACHE["nc"]

    res = run_bass_kernel_spmd(nc, in_maps, core_ids=list(range(N_CORES)))
    return np.concatenate([res.results[c]["out"] for c in range(N_CORES)], axis=0)


def _make_cand_table(table_size=16, ncand=16):
    """Interval table: row j lists integer candidates c with |c| mod 16 == j.
    [:, :ncand] = c, [:, ncand:] = c + 1; unused slots get a +1e9 sentinel."""
    lo = np.full((table_size, ncand), 1e9, np.float32)
    hi = np.full((table_size, ncand), 1e9, np.float32)
    half = ncand // 2
    for j in range(table_size):
        for k in range(half):
            c = j + table_size * k
            lo[j, k] = c
            hi[j, k] = c + 1
            m = j + table_size * k
            if m > 0:
                lo[j, half + k] = -m
                hi[j, half + k] = -m + 1
    return np.concatenate([lo, hi], axis=1)


from contextlib import ExitStack

import concourse.bass as bass
import concourse.mybir as mybir
import concourse.tile as tile
from concourse.masks import make_identity

P = 128
C = 256
H = W = 56
KS = 3
SPAN = KS * KS * C          # 2304
O = 512
TABLE = 16
BUCKET = 32
M_AUG = 5
NLOC = 2                    # images per core
HP = H + 2                  # 58 padded row stride
PADN = HP * HP              # 3364
RT = 7                      # output rows per row-tile
NT = H // RT                # 8 row-tiles per image
NPIX = RT * W               # 392
KCH = SPAN // P             # 18 span chunks
NCAND = 16                  # floor-interval candidates per hash bucket
NCG = 4                     # col-tile groups per PSUM pass
NG = NT // NCG              # 2 groups per image

f32 = mybir.dt.float32
f32r = mybir.dt.float32r
i32 = mybir.dt.int32
bf16 = mybir.dt.bfloat16
AF = mybir.ActivationFunctionType
ALU = mybir.AluOpType


def build(nc, mm_dtype="f32r", col_tile=True, reps=1, loop_reps=0, xload="pad_dma", fake_route=False, skip_mm=False, route_cut="", route_tt=False, route_noatail=False, xsplit=True):
    xs = nc.dram_tensor("xs", [NLOC, C, H, W], f32, kind="ExternalInput").ap()
    xq = nc.dram_tensor("xq", [C, 2, 2], f32, kind="ExternalInput").ap()
    av = nc.dram_tensor("av", [SPAN + M_AUG], f32, kind="ExternalInput").ap()
    kern = nc.dram_tensor("kern", [O, SPAN], f32, kind="ExternalInput").ap()
    tab = nc.dram_tensor("tab", [TABLE, BUCKET], i32, kind="ExternalInput").ap()
    cand = nc.dram_tensor("cand", [TABLE, 2 * NCAND], f32, kind="ExternalInput").ap()
    out = nc.dram_tensor("out", [NLOC, BUCKET, H, W], f32, kind="ExternalOutput").ap()

    with ExitStack() as ctx:
        tc = ctx.enter_context(tile.TileContext(nc))
        pools = dict(
            xpool=ctx.enter_context(tc.tile_pool(name="xpool", bufs=1)),
            spool=ctx.enter_context(tc.tile_pool(name="spool", bufs=1)),
            opool=ctx.enter_context(tc.tile_pool(name="opool", bufs=4)),
            pp_small=ctx.enter_context(tc.tile_pool(name="pp_small", bufs=2, space="PSUM")),
            pp_main=ctx.enter_context(tc.tile_pool(name="pp_main", bufs=4, space="PSUM")),
        )
        kw = dict(mm_dtype=mm_dtype, col_tile=col_tile, xload=xload,
                  fake_route=fake_route, skip_mm=skip_mm, route_cut=route_cut,
                  route_tt=route_tt, route_noatail=route_noatail, xsplit=xsplit)
        if loop_reps:
            with tc.For_i(0, loop_reps, 1):
                _build_body(nc, pools, xs, xq, av, kern, tab, cand, out, **kw)
        else:
            for _rep in range(reps):
                _build_body(nc, pools, xs, xq, av, kern, tab, cand, out, **kw)
    return nc


def _build_body(nc, pools, xs, xq, av, kern, tab, cand, out, mm_dtype, col_tile,
                xload="pad_dma", fake_route=False, skip_mm=False, route_cut="",
                route_tt=False, route_noatail=False, xsplit=True):
    x_dt = {"bf16": bf16, "f32r": f32r, "f32": f32}[mm_dtype]
    w_dt = x_dt
    if True:
        xpool = pools["xpool"]
        spool = pools["spool"]
        opool = pools["opool"]
        pp_small = pools["pp_small"]
        pp_main = pools["pp_main"]

        # ---- Stage A: hash / route / gather (query loads first: tiny, latency-critical) ----
        if not fake_route:
            # query patch, free layout (kki, ch); kki indexes (dy,dx) in {0,1}^2
            xq_sb = spool.tile([P, 8], f32, tag="xq")
            nc.sync.dma_start(
                out=xq_sb[:],
                in_=bass.AP(xq.tensor, 0, [[4, P], [1, 4], [2 * 2 * P, 2]]),
            )
            # a[1024:2304] as (p, kk-4, ch): one contiguous strided load
            asel = spool.tile([P, 10], f32, tag="asel")
            nc.sync.dma_start(
                out=asel[:],
                in_=bass.AP(av.tensor, 4 * C, [[1, P], [P, 10]]),
            )
            cand_sb = spool.tile([TABLE, 2 * NCAND], f32, tag="cand")
            nc.sync.dma_start(out=cand_sb[:], in_=cand[:])
            tab_i = spool.tile([TABLE, BUCKET], i32, tag="tabi")
            nc.sync.dma_start(out=tab_i[:], in_=tab[:])
            # a_tail partials for the augmented-query term
            s12 = spool.tile([P, 3], f32, tag="s12")
            nc.vector.memset(s12[:, 2:3], 0)
            nc.sync.dma_start(out=s12[0:M_AUG, 2:3], in_=av[SPAN:SPAN + M_AUG, None])
        # ---- Stage B: x -> padded SBUF tiles (issue DMAs first) ----
        xb = [[None, None], [None, None]]
        for n in range(NLOC):
            for ch in range(2):
                t = xpool.tile([P, PADN], x_dt, tag=f"xb{n}{ch}")
                tr = t[:].rearrange("p (r c) -> p r c", c=HP)
                # zero borders: top row, bottom row, left col, right col
                # (memset can't encode f32r; zero bits via an i32 view)
                trz = t[:].bitcast(i32).rearrange("p (r c) -> p r c", c=HP) if x_dt == f32r else tr
                nc.gpsimd.memset(trz[:, 0:1, :], 0)
                nc.gpsimd.memset(trz[:, HP - 1:HP, :], 0)
                nc.gpsimd.memset(trz[:, 1:HP - 1, 0:1], 0)
                nc.gpsimd.memset(trz[:, 1:HP - 1, HP - 1:HP], 0)
                src = xs[n, ch * P:(ch + 1) * P]
                dst = tr[:, 1:HP - 1, 1:HP - 1]
                if xload == "dve":
                    # contiguous f32 DMA at line rate, then engine reformat+cast;
                    # two row-halves so reformat overlaps the tail of the DMA
                    stg = xpool.tile([P, H * W], f32, tag=f"stg{n}{ch}")
                    sr = stg[:].rearrange("p (r c) -> p r c", c=W)
                    srcr = src
                    nsplit = 2 if xsplit else 1
                    hh = H // nsplit
                    for half in range(nsplit):
                        r0, r1 = half * hh, (half + 1) * hh
                        nc.sync.dma_start(out=sr[:, r0:r1, :], in_=srcr[:, r0:r1, :])
                        if (n + ch) % 2 == 0:
                            nc.vector.tensor_copy(out=dst[:, r0:r1, :], in_=sr[:, r0:r1, :])
                        else:
                            nc.scalar.activation(out=dst[:, r0:r1, :], in_=sr[:, r0:r1, :], func=AF.Copy)
                elif x_dt == f32:
                    nc.sync.dma_start(out=dst, in_=src)
                elif x_dt == f32r:
                    nc.sync.dma_start(out=dst, in_=src.bitcast(f32r))
                else:
                    nc.gpsimd.dma_start(out=dst, in_=src)  # casting DMA
                xb[n][ch] = t

        if fake_route:
            gt = spool.tile([P, KCH * BUCKET], w_dt, tag="gt")
            z = gt[:].bitcast(i32) if w_dt == f32r else gt[:]
            nc.vector.memset(z, 0)
            _stage_c(nc, pools, xb, gt, out, mm_dtype, col_tile, skip_mm)
            return

        tab_f = spool.tile([TABLE, BUCKET], f32, tag="tabf")
        nc.vector.tensor_copy(out=tab_f[:], in_=tab_i[:])

        # per-partition partials: s12[:,0]=xq.asel, s12[:,1]=xq.xq, s12[:,2]=a_tail (p<5)
        prod = spool.tile([P, 8], f32, tag="prod")
        asel_r = asel[:].rearrange("p (kk ch) -> p kk ch", ch=2)
        xq_r = xq_sb[:].rearrange("p (kk ch) -> p kk ch", ch=2)
        nc.vector.tensor_tensor(out=prod[:].rearrange("p (kk ch) -> p kk ch", ch=2)[:, 0:2, :],
                                in0=xq_r[:, 0:2, :], in1=asel_r[:, 0:2, :], op=ALU.mult)
        nc.vector.tensor_tensor(out=prod[:].rearrange("p (kk ch) -> p kk ch", ch=2)[:, 2:4, :],
                                in0=xq_r[:, 2:4, :], in1=asel_r[:, 3:5, :], op=ALU.mult)
        nc.vector.reduce_sum(s12[:, 0:1], prod[:], axis=mybir.AxisListType.X)
        nc.vector.tensor_tensor(out=prod[:], in0=xq_sb[:], in1=xq_sb[:], op=ALU.mult)
        nc.vector.reduce_sum(s12[:, 1:2], prod[:], axis=mybir.AxisListType.X)
        # cross-partition sums, replicated on 16 partitions: s_ps = ones.T @ s12
        ones16 = spool.tile([P, TABLE], f32, tag="ones16")
        nc.vector.memset(ones16[:], 1.0)
        s_ps = pp_small.tile([TABLE, 3], f32, space="PSUM", tag="pss")
        nc.tensor.matmul(out=s_ps[:], lhsT=ones16[:], rhs=s12[:], start=True, stop=True)
        s1_sb = spool.tile([TABLE, 1], f32, tag="s1_sb")
        nc.vector.tensor_copy(out=s1_sb[:], in_=s_ps[:, 0:1])
        if route_cut == "A":
            gt = spool.tile([P, KCH * BUCKET], w_dt, tag="gt")
            nc.vector.memset(gt[:], 0)
            _stage_c(nc, pools, xb, gt, out, mm_dtype, col_tile, skip_mm)
            return
        # onehot[j] = any_k( cand*nrm - 0.5*sa*nrm <= s1 < candhi*nrm - 0.5*sa*nrm )
        nrm = spool.tile([TABLE, 2], f32, tag="nrm")
        nc.scalar.activation(out=nrm[:, 0:1], in_=s_ps[:, 1:2], func=AF.Sqrt)
        nc.vector.scalar_tensor_tensor(
            out=nrm[:, 1:2], in0=s_ps[:, 2:3], scalar=-0.5, in1=nrm[:, 0:1],
            op0=ALU.mult, op1=ALU.mult)  # -0.5*sa*nrm
        bnd = spool.tile([TABLE, 2 * NCAND], f32, tag="bnd")
        nc.vector.tensor_scalar(
            out=bnd[:], in0=cand_sb[:], scalar1=nrm[:, 0:1], scalar2=nrm[:, 1:2],
            op0=ALU.mult, op1=ALU.add)  # cand*nrm - 0.5*sa*nrm
        cmp = spool.tile([TABLE, NCAND], f32, tag="cmp")
        nc.vector.tensor_scalar(
            out=cmp[:], in0=bnd[:, 0:NCAND], scalar1=s1_sb[:], scalar2=None,
            op0=ALU.is_le, op1=ALU.bypass)
        nc.vector.scalar_tensor_tensor(
            out=cmp[:], in0=bnd[:, NCAND:], scalar=s1_sb[:], in1=cmp[:],
            op0=ALU.is_gt, op1=ALU.mult)
        onehot = spool.tile([TABLE, 1], f32, tag="onehot")
        nc.vector.tensor_reduce(onehot[:], cmp[:], axis=mybir.AxisListType.X, op=ALU.add)
        if route_cut == "B":
            gt = spool.tile([P, KCH * BUCKET], w_dt, tag="gt")
            nc.vector.memset(gt[:], 0)
            _stage_c(nc, pools, xb, gt, out, mm_dtype, col_tile, skip_mm)
            return

        # selected row ids = tab_f.T @ onehot -> (32, 1)
        rows_ps = pp_small.tile([BUCKET, 1], f32, space="PSUM", tag="pss")
        nc.tensor.matmul(out=rows_ps[:], lhsT=tab_f[:], rhs=onehot[:], start=True, stop=True)
        rows_i = spool.tile([BUCKET, 1], i32, tag="rows_i")
        nc.vector.tensor_copy(out=rows_i[:], in_=rows_ps[:])
        if route_cut == "C":
            gt = spool.tile([P, KCH * BUCKET], w_dt, tag="gt")
            nc.vector.memset(gt[:], 0)
            _stage_c(nc, pools, xb, gt, out, mm_dtype, col_tile, skip_mm)
            return

        # gather the 32 selected kernel rows, casting to the matmul dtype
        ksel = spool.tile([BUCKET, SPAN], w_dt, tag="ksel")
        nc.gpsimd.indirect_dma_start(
            out=ksel[:], out_offset=None, in_=kern[:],
            in_offset=bass.IndirectOffsetOnAxis(ap=rows_i[:, :1], axis=0),
        )

        # transpose to GT (span-major): 18 chunks into one PSUM bank, one copy out
        ident = spool.tile([BUCKET, BUCKET], w_dt, tag="ident")
        make_identity(nc, ident[:])
        gt = spool.tile([P, KCH * BUCKET], w_dt, tag="gt")
        tp = pp_small.tile([P, KCH * BUCKET], w_dt, space="PSUM", tag="tp_all")
        for k in range(KCH):
            nc.tensor.transpose(out=tp[:, k * BUCKET:(k + 1) * BUCKET],
                                in_=ksel[:, k * P:(k + 1) * P], identity=ident[:])
        nc.vector.tensor_copy(out=gt[:], in_=tp[:])
        _stage_c(nc, pools, xb, gt, out, mm_dtype, col_tile, skip_mm)


def _stage_c(nc, pools, xb, gt, out, mm_dtype, col_tile, skip_mm):
    opool = pools["opool"]
    pp_main = pools["pp_main"]
    if skip_mm:
        return
    if True:
        # ---- Stage C: main matmuls ----
        # col-tile geometry: (#concurrent groups, partition pitch between groups)
        if not col_tile:
            ncg, pitch = 1, 32
        elif mm_dtype == "f32r" or mm_dtype == "f32":
            ncg, pitch = 2, 64   # fp32-family matmuls need 64-aligned dst partitions
        else:
            ncg, pitch = 4, 32
        ng = NT // ncg
        for n in range(NLOC):
            for g in range(ng):
                ps = pp_main.tile([(ncg - 1) * pitch + 32, 512], f32, space="PSUM", tag="ps")
                for k in range(KCH):
                    kk, ch = divmod(k, 2)
                    dy, dx = divmod(kk, KS)
                    lw = gt[:, k * BUCKET:(k + 1) * BUCKET]
                    xr = xb[n][ch][:].rearrange("p (r c) -> p r c", c=HP)
                    for cg in range(ncg):
                        t = g * ncg + cg
                        y0 = t * RT
                        rhs = xr[:, y0 + dy:y0 + dy + RT, dx:dx + W]
                        nc.tensor.matmul(
                            out=ps[pitch * cg:pitch * cg + 32, 0:NPIX],
                            lhsT=lw, rhs=rhs,
                            start=(k == 0), stop=(k == KCH - 1),
                            tile_position=((0, pitch * cg) if col_tile else None),
                            skip_group_check=True,
                        )
                for cg in range(ncg):
                    ob = opool.tile([32, NPIX], f32, tag="ob")
                    nc.scalar.activation(out=ob[:], in_=ps[pitch * cg:pitch * cg + 32, 0:NPIX], func=AF.Copy, scale=float(O // BUCKET))
                    t = g * ncg + cg
                    dst = out[n].rearrange("b (t r) w -> t b (r w)", r=RT)[t]
                    nc.sync.dma_start(out=dst, in_=ob[:])



# revision 2
# speedup vs baseline: 1.1464x; 1.1464x over previous
"""ALSH Conv2d (LSH-routed subset conv) as a Bass/Tile kernel on 8 TRN2 NeuronCores.

v2 over the session-1 baseline:
  * PSUM evacuation fused: one (128, 392) copy per (n, g) instead of 16
    per-cg (32, 392) copies; the O/BUCKET=16 output scale is folded into the
    transposed weight tile (ACT copy with scale), so evac copies are pure.
  * Output DMA fused: one 4-level-AP DMA per image (2 per pass) instead of 16.
  * x staging: contiguous f32 DMA halves into a 3-slot ping-pong pool, cast
    + padded-layout reformat alternating DVE/ACT per half.
  * Timing loop: runtime trip count (one compile serves correctness + both
    repeat points), 4x unrolled body with staggered_reset (no back-edge
    barrier/drain -> cross-iteration overlap) and a PE branch-prefetch hint.
"""
import numpy as np

N_CORES = 8
_NC_CACHE = {}


def kernel(x, kernels, a, table):
    import concourse.bacc as bacc
    from concourse.bass_utils import run_bass_kernel_spmd

    x = np.ascontiguousarray(np.asarray(x, dtype=np.float32))
    kernels = np.ascontiguousarray(np.asarray(kernels, dtype=np.float32))
    a = np.ascontiguousarray(np.asarray(a, dtype=np.float32))
    table = np.ascontiguousarray(np.asarray(table, dtype=np.int32))

    per = x.shape[0] // N_CORES
    xq = np.ascontiguousarray(x[0, :, 0:2, 0:2])
    cand = _make_cand_table()
    in_maps = [
        {"xs": x[c * per:(c + 1) * per], "xq": xq, "av": a, "kern": kernels,
         "tab": table, "cand": cand}
        for c in range(N_CORES)
    ]

    if "nc" not in _NC_CACHE:
        nc = bacc.Bacc("TRN2", target_bir_lowering=False, debug=False)
        build(nc)
        nc.compile()
        _NC_CACHE["nc"] = nc
    nc = _NC_CACHE["nc"]

    res = run_bass_kernel_spmd(nc, in_maps, core_ids=list(range(N_CORES)))
    return np.concatenate([res.results[c]["out"] for c in range(N_CORES)], axis=0)


def _make_cand_table(table_size=16, ncand=16):
    """Interval table: row j lists integer candidates c with |c| mod 16 == j.
    [:, :ncand] = c, [:, ncand:] = c + 1; unused slots get a +1e9 sentinel."""
    lo = np.full((table_size, ncand), 1e9, np.float32)
    hi = np.full((table_size, ncand), 1e9, np.float32)
    half = ncand // 2
    for j in range(table_size):
        for k in range(half):
            c = j + table_size * k
            lo[j, k] = c
            hi[j, k] = c + 1
            m = j + table_size * k
            if m > 0:
                lo[j, half + k] = -m
                hi[j, half + k] = -m + 1
    return np.concatenate([lo, hi], axis=1)


from contextlib import ExitStack

import concourse.bass as bass
import concourse.mybir as mybir
import concourse.tile as tile
from concourse.masks import make_identity

P = 128
C = 256
H = W = 56
KS = 3
SPAN = KS * KS * C          # 2304
O = 512
TABLE = 16
BUCKET = 32
M_AUG = 5
NLOC = 2                    # images per core
HP = H + 2                  # 58 padded row stride
PADN = HP * HP              # 3364
RT = 7                      # output rows per row-tile
NT = H // RT                # 8 row-tiles per image
NPIX = RT * W               # 392
KCH = SPAN // P             # 18 span chunks
NCAND = 16                  # floor-interval candidates per hash bucket
NCG = 4                     # col-tile groups per PSUM pass
NG = NT // NCG              # 2 groups per image

f32 = mybir.dt.float32
i32 = mybir.dt.int32
bf16 = mybir.dt.bfloat16
AF = mybir.ActivationFunctionType
ALU = mybir.AluOpType
ET = mybir.EngineType


def build(nc, loop_reps=0, runtime_reps=False, unroll=1, staggered=False,
          hints=(), out_fused=True, **_compat):
    xs = nc.dram_tensor("xs", [NLOC, C, H, W], f32, kind="ExternalInput").ap()
    xq = nc.dram_tensor("xq", [C, 2, 2], f32, kind="ExternalInput").ap()
    av = nc.dram_tensor("av", [SPAN + M_AUG], f32, kind="ExternalInput").ap()
    kern = nc.dram_tensor("kern", [O, SPAN], f32, kind="ExternalInput").ap()
    tab = nc.dram_tensor("tab", [TABLE, BUCKET], i32, kind="ExternalInput").ap()
    cand = nc.dram_tensor("cand", [TABLE, 2 * NCAND], f32, kind="ExternalInput").ap()
    out = nc.dram_tensor("out", [NLOC, BUCKET, H, W], f32, kind="ExternalOutput").ap()
    nrep = None
    if runtime_reps:
        nrep = nc.dram_tensor("nrep", [1], i32, kind="ExternalInput").ap()

    with ExitStack() as ctx:
        tc = ctx.enter_context(tile.TileContext(nc))
        pools = dict(
            xpool=ctx.enter_context(tc.tile_pool(name="xpool", bufs=2)),
            stgpool=ctx.enter_context(tc.tile_pool(name="stgpool", bufs=3)),
            spool=ctx.enter_context(tc.tile_pool(name="spool", bufs=2)),
            opool=ctx.enter_context(tc.tile_pool(name="opool", bufs=2)),
            pp_small=ctx.enter_context(tc.tile_pool(name="pp_small", bufs=2, space="PSUM")),
            pp_main=ctx.enter_context(tc.tile_pool(name="pp_main", bufs=4, space="PSUM")),
        )
        kw = dict(out_fused=out_fused)
        if runtime_reps:
            trips_sb = pools["spool"].tile([1, 1], i32, tag="nrep")
            nc.sync.dma_start(out=trips_sb[:], in_=nrep[0:1, None])
            trips = nc.values_load(trips_sb[0:1, 0:1], min_val=0, max_val=1 << 20)
            with tc.For_i(0, trips, 1, staggered_reset=staggered, hint_engines=tuple(hints)):
                for u in range(unroll):
                    if u and staggered:
                        tc.stage_boundary()
                    _build_body(nc, pools, xs, xq, av, kern, tab, cand, out, **kw)
        elif loop_reps:
            with tc.For_i(0, loop_reps, 1, staggered_reset=staggered, hint_engines=tuple(hints)):
                for u in range(unroll):
                    if u and staggered:
                        tc.stage_boundary()
                    _build_body(nc, pools, xs, xq, av, kern, tab, cand, out, **kw)
        else:
            _build_body(nc, pools, xs, xq, av, kern, tab, cand, out, **kw)
    return nc


def _build_body(nc, pools, xs, xq, av, kern, tab, cand, out, out_fused=True):
    xpool = pools["xpool"]
    stgpool = pools["stgpool"]
    spool = pools["spool"]
    opool = pools["opool"]
    pp_small = pools["pp_small"]
    pp_main = pools["pp_main"]

    # ---- Stage A: hash / route / gather (query loads first: tiny, latency-critical) ----
    # query patch, free layout (kki, ch); kki indexes (dy,dx) in {0,1}^2
    xq_sb = spool.tile([P, 8], f32, tag="xq")
    nc.sync.dma_start(
        out=xq_sb[:],
        in_=bass.AP(xq.tensor, 0, [[4, P], [1, 4], [2 * 2 * P, 2]]),
    )
    # a[1024:2304] as (p, kk-4, ch): one contiguous strided load
    asel = spool.tile([P, 10], f32, tag="asel")
    nc.sync.dma_start(
        out=asel[:],
        in_=bass.AP(av.tensor, 4 * C, [[1, P], [P, 10]]),
    )
    cand_sb = spool.tile([TABLE, 2 * NCAND], f32, tag="cand")
    nc.sync.dma_start(out=cand_sb[:], in_=cand[:])
    tab_i = spool.tile([TABLE, BUCKET], i32, tag="tabi")
    nc.sync.dma_start(out=tab_i[:], in_=tab[:])
    # a_tail partials for the augmented-query term
    s12 = spool.tile([P, 3], f32, tag="s12")
    nc.vector.memset(s12[:, 2:3], 0)
    nc.sync.dma_start(out=s12[0:M_AUG, 2:3], in_=av[SPAN:SPAN + M_AUG, None])

    # ---- Stage B: x -> padded SBUF tiles (issue DMAs first) ----
    xb = [[None, None], [None, None]]
    half_idx = 0
    for n in range(NLOC):
        for ch in range(2):
            t = xpool.tile([P, PADN], bf16, tag=f"xb{n}{ch}")
            tr = t[:].rearrange("p (r c) -> p r c", c=HP)
            # zero borders: top row, bottom row, left col, right col
            nc.gpsimd.memset(tr[:, 0:1, :], 0)
            nc.gpsimd.memset(tr[:, HP - 1:HP, :], 0)
            nc.gpsimd.memset(tr[:, 1:HP - 1, 0:1], 0)
            nc.gpsimd.memset(tr[:, 1:HP - 1, HP - 1:HP], 0)
            src = xs[n, ch * P:(ch + 1) * P]
            dst = tr[:, 1:HP - 1, 1:HP - 1]
            # contiguous f32 DMA at line rate, then engine reformat+cast;
            # row-halves through a shared 3-slot pool so reformat overlaps DMA
            hh = H // 2
            for half in range(2):
                r0, r1 = half * hh, (half + 1) * hh
                stg = stgpool.tile([P, hh * W], f32, tag="stg")
                sr = stg[:].rearrange("p (r c) -> p r c", c=W)
                nc.sync.dma_start(out=sr[:], in_=src[:, r0:r1, :])
                if half_idx % 2 == 0:
                    nc.vector.tensor_copy(out=dst[:, r0:r1, :], in_=sr[:])
                else:
                    nc.scalar.activation(out=dst[:, r0:r1, :], in_=sr[:], func=AF.Copy)
                half_idx += 1
            xb[n][ch] = t

    tab_f = spool.tile([TABLE, BUCKET], f32, tag="tabf")
    nc.vector.tensor_copy(out=tab_f[:], in_=tab_i[:])

    # per-partition partials: s12[:,0]=xq.asel, s12[:,1]=xq.xq, s12[:,2]=a_tail (p<5)
    prod = spool.tile([P, 8], f32, tag="prod")
    asel_r = asel[:].rearrange("p (kk ch) -> p kk ch", ch=2)
    xq_r = xq_sb[:].rearrange("p (kk ch) -> p kk ch", ch=2)
    nc.vector.tensor_tensor(out=prod[:].rearrange("p (kk ch) -> p kk ch", ch=2)[:, 0:2, :],
                            in0=xq_r[:, 0:2, :], in1=asel_r[:, 0:2, :], op=ALU.mult)
    nc.vector.tensor_tensor(out=prod[:].rearrange("p (kk ch) -> p kk ch", ch=2)[:, 2:4, :],
                            in0=xq_r[:, 2:4, :], in1=asel_r[:, 3:5, :], op=ALU.mult)
    nc.vector.reduce_sum(s12[:, 0:1], prod[:], axis=mybir.AxisListType.X)
    nc.vector.tensor_tensor(out=prod[:], in0=xq_sb[:], in1=xq_sb[:], op=ALU.mult)
    nc.vector.reduce_sum(s12[:, 1:2], prod[:], axis=mybir.AxisListType.X)
    # cross-partition sums, replicated on 16 partitions: s_ps = ones.T @ s12
    ones16 = spool.tile([P, TABLE], f32, tag="ones16")
    nc.vector.memset(ones16[:], 1.0)
    s_ps = pp_small.tile([TABLE, 3], f32, space="PSUM", tag="pss")
    nc.tensor.matmul(out=s_ps[:], lhsT=ones16[:], rhs=s12[:], start=True, stop=True)
    s1_sb = spool.tile([TABLE, 1], f32, tag="s1_sb")
    nc.vector.tensor_copy(out=s1_sb[:], in_=s_ps[:, 0:1])
    # onehot[j] = any_k( cand*nrm - 0.5*sa*nrm <= s1 < candhi*nrm - 0.5*sa*nrm )
    nrm = spool.tile([TABLE, 2], f32, tag="nrm")
    nc.scalar.activation(out=nrm[:, 0:1], in_=s_ps[:, 1:2], func=AF.Sqrt)
    nc.vector.scalar_tensor_tensor(
        out=nrm[:, 1:2], in0=s_ps[:, 2:3], scalar=-0.5, in1=nrm[:, 0:1],
        op0=ALU.mult, op1=ALU.mult)  # -0.5*sa*nrm
    bnd = spool.tile([TABLE, 2 * NCAND], f32, tag="bnd")
    nc.vector.tensor_scalar(
        out=bnd[:], in0=cand_sb[:], scalar1=nrm[:, 0:1], scalar2=nrm[:, 1:2],
        op0=ALU.mult, op1=ALU.add)  # cand*nrm - 0.5*sa*nrm
    cmp = spool.tile([TABLE, NCAND], f32, tag="cmp")
    nc.vector.tensor_scalar(
        out=cmp[:], in0=bnd[:, 0:NCAND], scalar1=s1_sb[:], scalar2=None,
        op0=ALU.is_le, op1=ALU.bypass)
    nc.vector.scalar_tensor_tensor(
        out=cmp[:], in0=bnd[:, NCAND:], scalar=s1_sb[:], in1=cmp[:],
        op0=ALU.is_gt, op1=ALU.mult)
    onehot = spool.tile([TABLE, 1], f32, tag="onehot")
    nc.vector.tensor_reduce(onehot[:], cmp[:], axis=mybir.AxisListType.X, op=ALU.add)

    # selected row ids = tab_f.T @ onehot -> (32, 1)
    rows_ps = pp_small.tile([BUCKET, 1], f32, space="PSUM", tag="pss")
    nc.tensor.matmul(out=rows_ps[:], lhsT=tab_f[:], rhs=onehot[:], start=True, stop=True)
    rows_i = spool.tile([BUCKET, 1], i32, tag="rows_i")
    nc.vector.tensor_copy(out=rows_i[:], in_=rows_ps[:])

    # gather the 32 selected kernel rows, casting to the matmul dtype
    ksel = spool.tile([BUCKET, SPAN], bf16, tag="ksel")
    nc.gpsimd.indirect_dma_start(
        out=ksel[:], out_offset=None, in_=kern[:],
        in_offset=bass.IndirectOffsetOnAxis(ap=rows_i[:, :1], axis=0),
    )

    # transpose to GT (span-major): 18 chunks into one PSUM bank, one copy out.
    # The O/BUCKET=16 output scale is folded into the weights here.
    ident = spool.tile([BUCKET, BUCKET], bf16, tag="ident")
    make_identity(nc, ident[:])
    gt = spool.tile([P, KCH * BUCKET], bf16, tag="gt")
    tp = pp_small.tile([P, KCH * BUCKET], bf16, space="PSUM", tag="tp_all")
    for k in range(KCH):
        nc.tensor.transpose(out=tp[:, k * BUCKET:(k + 1) * BUCKET],
                            in_=ksel[:, k * P:(k + 1) * P], identity=ident[:])
    nc.scalar.activation(out=gt[:], in_=tp[:], func=AF.Copy, scale=float(O // BUCKET))

    # ---- Stage C: main matmuls (4-way PE column tiling, 32-wide groups) ----
    for n in range(NLOC):
        ob = opool.tile([P, NG * NPIX], f32, tag=f"ob{n}")
        for g in range(NG):
            ps = pp_main.tile([P, 512], f32, space="PSUM", tag="ps")
            for k in range(KCH):
                kk, ch = divmod(k, 2)
                dy, dx = divmod(kk, KS)
                lw = gt[:, k * BUCKET:(k + 1) * BUCKET]
                xr = xb[n][ch][:].rearrange("p (r c) -> p r c", c=HP)
                for cg in range(NCG):
                    t = g * NCG + cg
                    y0 = t * RT
                    rhs = xr[:, y0 + dy:y0 + dy + RT, dx:dx + W]
                    nc.tensor.matmul(
                        out=ps[32 * cg:32 * cg + 32, 0:NPIX],
                        lhsT=lw, rhs=rhs,
                        start=(k == 0), stop=(k == KCH - 1),
                        tile_position=(0, 32 * cg),
                        skip_group_check=True,
                    )
            # fused evacuation: all 4 col groups in one copy (scale already in gt)
            if g % 2 == 0:
                nc.vector.tensor_copy(out=ob[:, g * NPIX:(g + 1) * NPIX], in_=ps[:, 0:NPIX])
            else:
                nc.scalar.activation(out=ob[:, g * NPIX:(g + 1) * NPIX], in_=ps[:, 0:NPIX], func=AF.Copy)
        if out_fused:
            # partition p = cg*32 + b, free f = g*392 + j  ->
            # out[n, b, (g*4+cg)*392 + j]
            dst = bass.AP(out.tensor, n * BUCKET * H * W,
                          [[NPIX, NCG], [H * W, BUCKET], [NCG * NPIX, NG], [1, NPIX]])
            nc.sync.dma_start(out=dst, in_=ob[:])
        else:
            for g in range(NG):
                for cg in range(NCG):
                    t = g * NCG + cg
                    dstp = out[n].rearrange("b (t r) w -> t b (r w)", r=RT)[t]
                    nc.sync.dma_start(
                        out=dstp, in_=ob[32 * cg:32 * cg + 32, g * NPIX:(g + 1) * NPIX])
